# revision 19
# baseline (speedup 1.0000x reference)
"""Trainium2 Bass kernel for nn_AttentionMemoryBank.

Math (forward; mask/stop_gradient is identity in forward):
    xf     = x.reshape(B, K, N)                 # B=8, K=256, N=4096
    logits = einsum('km,bkn->bmn', mem[0], xf) / sqrt(K)   # (B, M=1024, N)
    attn   = softmax(logits, axis=1)            # over M
    out    = einsum('km,bmn->bkn', mem[0], attn).reshape(B, K, 64, 64)
    return (out, logits)

Distribution: data-parallel over batch — one batch element per NeuronCore
(8 cores), memory bank replicated.

Per-core kernel (bf16 matmuls, f32 PSUM accumulation; exp without
max-subtraction — logits ~ N(0,1) so exp is numerically safe):
  stage 1: logits tile (128m x NCHUNK) = memKM.T @ x      (PE)
           logits_sb  = PSUM * 1/16                       (DVE) -> HBM
           expl       = exp(PSUM * 1/16)  bf16            (ACT)
  stage 2: per 128-wide n-tile:
           O (128n x 257) = sum_mt expl_mt[:, nt].T @ [memT | 1]  (PE)
             -> O[:, :256] = unnormalized out.T, O[:, 256] = softmax denom
           outT = O[:, :256] * reciprocal(O[:, 256])      (DVE) -> HBM
Host: transposes per-core outT (4096,256) -> (256,64,64) while stacking.
"""

import sys

if "/opt/trn_rl_repo" not in sys.path:
    sys.path.insert(0, "/opt/trn_rl_repo")

import numpy as np

B = 8
K = 256
HH = 64
WW = 64
N = HH * WW          # 4096
M = 1024             # mem bank size
NCORES = 8
NCHUNK = 512         # n-columns per stage-1 matmul (PSUM bank = 512 f32)
SCALE = 1.0 / 16.0   # 1/sqrt(K)

KT = K // 128        # 2 contraction tiles for stage 1
MT = M // 128        # 8 logits partition tiles / stage-2 contraction tiles
NCH = N // NCHUNK    # stage-1 chunks per row
NT = N // 128        # stage-2 n-tiles
OB = 4               # outT n-tiles batched per DMA

_cache = {}


def _build():
    from concourse import bacc
    import concourse.tile as tile
    from concourse import mybir
    from contextlib import ExitStack

    f32 = mybir.dt.float32
    bf16 = mybir.dt.bfloat16

    nc = bacc.Bacc("TRN2", target_bir_lowering=False, debug=False,
                   num_devices=NCORES)

    x_d = nc.dram_tensor("x", [K, N], f32, kind="ExternalInput").ap()
    mkm_d = nc.dram_tensor("mem_km", [K, M], bf16, kind="ExternalInput").ap()
    mt1_d = nc.dram_tensor("mem_t1", [M, K + 1], bf16, kind="ExternalInput").ap()
    # outputs are stored bf16 on device (halves write traffic; the host
    # upcasts) — quantization adds ~1e-3 rel err, small next to the bf16
    # matmul error and far inside tolerance
    lg_d = nc.dram_tensor("logits", [M, N], bf16, kind="ExternalOutput").ap()
    ot_d = nc.dram_tensor("outT", [N, K], bf16, kind="ExternalOutput").ap()

    with tile.TileContext(nc) as tc, ExitStack() as ctx:
        const = ctx.enter_context(tc.tile_pool(name="const", bufs=1))

        x_sb = const.tile([128, KT, N], bf16)
        mkm_sb = const.tile([128, KT, M], bf16)
        mt1_sb = const.tile([128, MT, K + 1], bf16)
        x_re = x_d.rearrange("(t p) n -> p t n", p=128)
        # first x block rides HWDGE as f32 (starts earlier than SWDGE) and
        # DVE casts it; the rest streams via SWDGE with the f32->bf16 cast
        # fused into the DMA. mem bank loads share the HWDGE path.
        xst_pool = ctx.enter_context(tc.tile_pool(name="xstage", bufs=1))
        xst = xst_pool.tile([128, KT, 512], f32)
        nc.sync.dma_start(out=xst, in_=x_re[:, :, 0:512])
        nc.sync.dma_start(out=mkm_sb,
                          in_=mkm_d.rearrange("(t p) m -> p t m", p=128))
        nc.vector.tensor_copy(x_sb[:, :, 0:512], xst)
        nc.sync.dma_start(out=mt1_sb,
                          in_=mt1_d.rearrange("(t p) c -> p t c", p=128))
        x_edges = [512, 1024, 2048, 3072, 4096]
        for lo, hi in zip(x_edges[:-1], x_edges[1:]):
            nc.gpsimd.dma_start(out=x_sb[:, :, lo:hi], in_=x_re[:, :, lo:hi])

        psumL = ctx.enter_context(tc.tile_pool(name="psumL", bufs=3, space="PSUM"))
        psumO = ctx.enter_context(tc.tile_pool(name="psumO", bufs=2, space="PSUM"))
        lpool = ctx.enter_context(tc.tile_pool(name="lsb", bufs=2))
        epool = ctx.enter_context(tc.tile_pool(name="expl", bufs=1))
        opool = ctx.enter_context(tc.tile_pool(name="osb", bufs=2))
        rpool = ctx.enter_context(tc.tile_pool(name="rcp", bufs=4))

        # full exp(logits) stays resident: 8 x (128, 4096) bf16 = 64KB/partition
        expl = epool.tile([128, MT, N], bf16)
        ot_re = ot_d.rearrange("(g p) k -> p g k", p=128)

        HALVES = 2
        HN = N // HALVES          # columns per half
        HC = HN // NCHUNK         # stage-1 chunks per half
        HT = NT // HALVES         # stage-2 n-tiles per half

        LW = 2 * NCHUNK           # PSUM L spans 2 banks -> halves DVE/ACT ops
        NB = N // LW              # interleave blocks
        lsb = {}
        for h in range(HALVES):
            lsb_h = lpool.tile([128, MT, HN], bf16, tag="lsb")
            lsb[h] = lsb_h
        # Full interleave: each LW-wide block does stage 1 for all MT rows,
        # then immediately stage 2 for its LW//128 n-tiles — PE alternates
        # between the two matmul shapes with no phase barrier.
        for cc in range(NB):
            base = cc * LW
            h = (cc * LW) // HN
            nr = slice(base - h * HN, base - h * HN + LW)
            for mt in range(MT):
                ms = slice(mt * 128, (mt + 1) * 128)
                L = psumL.tile([128, LW], f32, tag="L")
                for sub in range(LW // NCHUNK):
                    ns = slice(base + sub * NCHUNK, base + (sub + 1) * NCHUNK)
                    ps = slice(sub * NCHUNK, (sub + 1) * NCHUNK)
                    for kt in range(KT):
                        nc.tensor.matmul(L[:, ps], mkm_sb[:, kt, ms],
                                         x_sb[:, kt, ns],
                                         start=(kt == 0), stop=(kt == KT - 1))
                nc.vector.tensor_scalar_mul(lsb[h][:, mt, nr], L, SCALE)
                nc.scalar.activation(expl[:, mt, base:base + LW], L,
                                     mybir.ActivationFunctionType.Exp,
                                     scale=SCALE)
            if (cc + 1) * LW % HN == 0:
                hs = slice(h * HN, (h + 1) * HN)
                for mt in range(MT):
                    nc.sync.dma_start(out=lg_d[mt * 128:(mt + 1) * 128, hs],
                                      in_=lsb[h][:, mt, :])
            # ---- stage 2 for this block's n-tiles ----
            for g in range(LW // 128 // OB):
                osb = opool.tile([128, OB, K], bf16, tag="osb")
                for t in range(OB):
                    nt = cc * (LW // 128) + g * OB + t
                    ts_ = slice(nt * 128, (nt + 1) * 128)
                    O = psumO.tile([128, K + 1], f32, tag="O")
                    for mt in range(MT):
                        nc.tensor.matmul(O, expl[:, mt, ts_], mt1_sb[:, mt, :],
                                         start=(mt == 0), stop=(mt == MT - 1))
                    r = rpool.tile([128, 1], f32, tag="rcp")
                    nc.vector.reciprocal(r, O[:, K:K + 1])
                    # alternate the normalize between ACT and DVE to balance
                    if t % 2 == 0:
                        nc.scalar.mul(osb[:, t, :], O[:, 0:K], r)
                    else:
                        nc.vector.tensor_scalar_mul(osb[:, t, :], O[:, 0:K], r)
                g0 = cc * (LW // 128) + g * OB
                nc.sync.dma_start(out=ot_re[:, g0:g0 + OB, :], in_=osb)

    nc.compile()
    return nc


def _get_nc():
    if "nc" not in _cache:
        _cache["nc"] = _build()
    return _cache["nc"]


def _run(x, mem, trace=False, **kwargs):
    import ml_dtypes
    from concourse.bass_utils import run_bass_kernel_spmd

    nc = _get_nc()
    x = np.asarray(x, dtype=np.float32).reshape(B, K, N)
    mem0 = np.asarray(mem, dtype=np.float32)[0]                    # (K, M)
    mkm = mem0.astype(ml_dtypes.bfloat16)
    mt1 = np.concatenate(
        [mem0.T, np.ones((M, 1), np.float32)], axis=1
    ).astype(ml_dtypes.bfloat16)                                   # (M, K+1)
    in_maps = [
        {"x": np.ascontiguousarray(x[b]), "mem_km": mkm, "mem_t1": mt1}
        for b in range(B)
    ]
    return run_bass_kernel_spmd(nc, in_maps, core_ids=list(range(NCORES)),
                                trace=trace, **kwargs)


def kernel(x, mask, mem):
    res = _run(x, mem, trace=False)
    logits = np.stack(
        [res.results[b]["logits"].astype(np.float32) for b in range(B)]
    )
    out = np.stack(
        [res.results[b]["outT"].astype(np.float32).T.reshape(K, HH, WW)
         for b in range(B)]
    )
    return out, logits


# revision 20
# speedup vs baseline: 1.0091x; 1.0091x over previous
"""Trainium2 Bass kernel for nn_AttentionMemoryBank.

Math (forward; mask/stop_gradient is identity in forward):
    xf     = x.reshape(B, K, N)                 # B=8, K=256, N=4096
    logits = einsum('km,bkn->bmn', mem[0], xf) / sqrt(K)   # (B, M=1024, N)
    attn   = softmax(logits, axis=1)            # over M
    out    = einsum('km,bmn->bkn', mem[0], attn).reshape(B, K, 64, 64)
    return (out, logits)

Distribution: data-parallel over batch — one batch element per NeuronCore
(8 cores), memory bank replicated.

Per-core kernel (bf16 matmuls, f32 PSUM accumulation; exp without
max-subtraction — logits ~ N(0,1) so exp is numerically safe):
  stage 1: logits tile (128m x NCHUNK) = memKM.T @ x      (PE)
           logits_sb  = PSUM * 1/16                       (DVE) -> HBM
           expl       = exp(PSUM * 1/16)  bf16            (ACT)
  stage 2: per 128-wide n-tile:
           O (128n x 257) = sum_mt expl_mt[:, nt].T @ [memT | 1]  (PE)
             -> O[:, :256] = unnormalized out.T, O[:, 256] = softmax denom
           outT = O[:, :256] * reciprocal(O[:, 256])      (DVE) -> HBM
Host: transposes per-core outT (4096,256) -> (256,64,64) while stacking.
"""

import sys

if "/opt/trn_rl_repo" not in sys.path:
    sys.path.insert(0, "/opt/trn_rl_repo")

import numpy as np

B = 8
K = 256
HH = 64
WW = 64
N = HH * WW          # 4096
M = 1024             # mem bank size
NCORES = 8
NCHUNK = 512         # n-columns per stage-1 matmul (PSUM bank = 512 f32)
SCALE = 1.0 / 16.0   # 1/sqrt(K)

KT = K // 128        # 2 contraction tiles for stage 1
MT = M // 128        # 8 logits partition tiles / stage-2 contraction tiles
NCH = N // NCHUNK    # stage-1 chunks per row
NT = N // 128        # stage-2 n-tiles
OB = 4               # outT n-tiles batched per DMA

_cache = {}


def _build():
    from concourse import bacc
    import concourse.tile as tile
    from concourse import mybir
    from contextlib import ExitStack

    f32 = mybir.dt.float32
    bf16 = mybir.dt.bfloat16

    nc = bacc.Bacc("TRN2", target_bir_lowering=False, debug=False,
                   num_devices=NCORES)

    x_d = nc.dram_tensor("x", [K, N], f32, kind="ExternalInput").ap()
    mkm_d = nc.dram_tensor("mem_km", [K, M], bf16, kind="ExternalInput").ap()
    mt1_d = nc.dram_tensor("mem_t1", [M, K + 1], bf16, kind="ExternalInput").ap()
    # outputs are stored bf16 on device (halves write traffic; the host
    # upcasts) — quantization adds ~1e-3 rel err, small next to the bf16
    # matmul error and far inside tolerance
    lg_d = nc.dram_tensor("logits", [M, N], bf16, kind="ExternalOutput").ap()
    ot_d = nc.dram_tensor("outT", [N, K], bf16, kind="ExternalOutput").ap()

    with tile.TileContext(nc) as tc, ExitStack() as ctx:
        const = ctx.enter_context(tc.tile_pool(name="const", bufs=1))

        x_sb = const.tile([128, KT, N], bf16)
        mkm_sb = const.tile([128, KT, M], bf16)
        mt1_sb = const.tile([128, MT, K + 1], bf16)
        x_re = x_d.rearrange("(t p) n -> p t n", p=128)
        # first x block rides HWDGE as f32 (starts earlier than SWDGE) and
        # DVE casts it; the rest streams via SWDGE with the f32->bf16 cast
        # fused into the DMA. mem bank loads share the HWDGE path.
        xst_pool = ctx.enter_context(tc.tile_pool(name="xstage", bufs=1))
        xst = xst_pool.tile([128, KT, 512], f32)
        mkm_re = mkm_d.rearrange("(t p) m -> p t m", p=128)
        # HWDGE order is what the first matmul waits on: tiny first slice of
        # the memory bank, then the f32-staged first x block, then the rest
        nc.sync.dma_start(out=mkm_sb[:, :, 0:128], in_=mkm_re[:, :, 0:128])
        nc.sync.dma_start(out=xst, in_=x_re[:, :, 0:512])
        nc.vector.tensor_copy(x_sb[:, :, 0:512], xst)
        nc.sync.dma_start(out=mkm_sb[:, :, 128:M], in_=mkm_re[:, :, 128:M])
        nc.sync.dma_start(out=mt1_sb,
                          in_=mt1_d.rearrange("(t p) c -> p t c", p=128))
        x_edges = [512, 1024, 2048, 3072, 4096]
        for lo, hi in zip(x_edges[:-1], x_edges[1:]):
            nc.gpsimd.dma_start(out=x_sb[:, :, lo:hi], in_=x_re[:, :, lo:hi])

        psumL = ctx.enter_context(tc.tile_pool(name="psumL", bufs=3, space="PSUM"))
        psumO = ctx.enter_context(tc.tile_pool(name="psumO", bufs=2, space="PSUM"))
        lpool = ctx.enter_context(tc.tile_pool(name="lsb", bufs=2))
        epool = ctx.enter_context(tc.tile_pool(name="expl", bufs=1))
        opool = ctx.enter_context(tc.tile_pool(name="osb", bufs=2))
        rpool = ctx.enter_context(tc.tile_pool(name="rcp", bufs=4))

        # full exp(logits) stays resident: 8 x (128, 4096) bf16 = 64KB/partition
        expl = epool.tile([128, MT, N], bf16)
        ot_re = ot_d.rearrange("(g p) k -> p g k", p=128)

        HALVES = 2
        HN = N // HALVES          # columns per half
        HC = HN // NCHUNK         # stage-1 chunks per half
        HT = NT // HALVES         # stage-2 n-tiles per half

        LW = 2 * NCHUNK           # PSUM L spans 2 banks -> halves DVE/ACT ops
        NB = N // LW              # interleave blocks
        lsb = {}
        for h in range(HALVES):
            lsb_h = lpool.tile([128, MT, HN], bf16, tag="lsb")
            lsb[h] = lsb_h
        # Full interleave: each LW-wide block does stage 1 for all MT rows,
        # then immediately stage 2 for its LW//128 n-tiles — PE alternates
        # between the two matmul shapes with no phase barrier.
        for cc in range(NB):
            base = cc * LW
            h = (cc * LW) // HN
            nr = slice(base - h * HN, base - h * HN + LW)
            for mt in range(MT):
                ms = slice(mt * 128, (mt + 1) * 128)
                L = psumL.tile([128, LW], f32, tag="L")
                for sub in range(LW // NCHUNK):
                    ns = slice(base + sub * NCHUNK, base + (sub + 1) * NCHUNK)
                    ps = slice(sub * NCHUNK, (sub + 1) * NCHUNK)
                    for kt in range(KT):
                        nc.tensor.matmul(L[:, ps], mkm_sb[:, kt, ms],
                                         x_sb[:, kt, ns],
                                         start=(kt == 0), stop=(kt == KT - 1))
                nc.vector.tensor_scalar_mul(lsb[h][:, mt, nr], L, SCALE)
                nc.scalar.activation(expl[:, mt, base:base + LW], L,
                                     mybir.ActivationFunctionType.Exp,
                                     scale=SCALE)
            if (cc + 1) * LW % HN == 0:
                hs = slice(h * HN, (h + 1) * HN)
                for mt in range(MT):
                    nc.sync.dma_start(out=lg_d[mt * 128:(mt + 1) * 128, hs],
                                      in_=lsb[h][:, mt, :])
            # ---- stage 2 for this block's n-tiles ----
            for g in range(LW // 128 // OB):
                osb = opool.tile([128, OB, K], bf16, tag="osb")
                for t in range(OB):
                    nt = cc * (LW // 128) + g * OB + t
                    ts_ = slice(nt * 128, (nt + 1) * 128)
                    O = psumO.tile([128, K + 1], f32, tag="O")
                    for mt in range(MT):
                        nc.tensor.matmul(O, expl[:, mt, ts_], mt1_sb[:, mt, :],
                                         start=(mt == 0), stop=(mt == MT - 1))
                    r = rpool.tile([128, 1], f32, tag="rcp")
                    nc.vector.reciprocal(r, O[:, K:K + 1])
                    # alternate the normalize between ACT and DVE to balance
                    if t % 2 == 0:
                        nc.scalar.mul(osb[:, t, :], O[:, 0:K], r)
                    else:
                        nc.vector.tensor_scalar_mul(osb[:, t, :], O[:, 0:K], r)
                g0 = cc * (LW // 128) + g * OB
                nc.sync.dma_start(out=ot_re[:, g0:g0 + OB, :], in_=osb)

    nc.compile()
    return nc


def _get_nc():
    if "nc" not in _cache:
        _cache["nc"] = _build()
    return _cache["nc"]


def _run(x, mem, trace=False, **kwargs):
    import ml_dtypes
    from concourse.bass_utils import run_bass_kernel_spmd

    nc = _get_nc()
    x = np.asarray(x, dtype=np.float32).reshape(B, K, N)
    mem0 = np.asarray(mem, dtype=np.float32)[0]                    # (K, M)
    mkm = mem0.astype(ml_dtypes.bfloat16)
    mt1 = np.concatenate(
        [mem0.T, np.ones((M, 1), np.float32)], axis=1
    ).astype(ml_dtypes.bfloat16)                                   # (M, K+1)
    in_maps = [
        {"x": np.ascontiguousarray(x[b]), "mem_km": mkm, "mem_t1": mt1}
        for b in range(B)
    ]
    return run_bass_kernel_spmd(nc, in_maps, core_ids=list(range(NCORES)),
                                trace=trace, **kwargs)


def kernel(x, mask, mem):
    res = _run(x, mem, trace=False)
    logits = np.stack(
        [res.results[b]["logits"].astype(np.float32) for b in range(B)]
    )
    out = np.stack(
        [res.results[b]["outT"].astype(np.float32).T.reshape(K, HH, WW)
         for b in range(B)]
    )
    return out, logits


# revision 24
# speedup vs baseline: 1.0538x; 1.0443x over previous
"""Trainium2 Bass kernel for nn_AttentionMemoryBank.

Math (forward; mask/stop_gradient is identity in forward):
    xf     = x.reshape(B, K, N)                 # B=8, K=256, N=4096
    logits = einsum('km,bkn->bmn', mem[0], xf) / sqrt(K)   # (B, M=1024, N)
    attn   = softmax(logits, axis=1)            # over M
    out    = einsum('km,bmn->bkn', mem[0], attn).reshape(B, K, 64, 64)
    return (out, logits)

Distribution: data-parallel over batch — one batch element per NeuronCore
(8 cores), memory bank replicated.

Per-core kernel (bf16 matmuls, f32 PSUM accumulation; exp without
max-subtraction — logits ~ N(0,1) so exp is numerically safe):
  stage 1: logits tile (128m x NCHUNK) = memKM.T @ x      (PE)
           logits_sb  = PSUM * 1/16                       (DVE) -> HBM
           expl       = exp(PSUM * 1/16)  bf16            (ACT)
  stage 2: per 128-wide n-tile:
           O (128n x 257) = sum_mt expl_mt[:, nt].T @ [memT | 1]  (PE)
             -> O[:, :256] = unnormalized out.T, O[:, 256] = softmax denom
           outT = O[:, :256] * reciprocal(O[:, 256])      (DVE) -> HBM
Host: transposes per-core outT (4096,256) -> (256,64,64) while stacking.
"""

import sys

if "/opt/trn_rl_repo" not in sys.path:
    sys.path.insert(0, "/opt/trn_rl_repo")

import numpy as np

B = 8
K = 256
HH = 64
WW = 64
N = HH * WW          # 4096
M = 1024             # mem bank size
NCORES = 8
NCHUNK = 512         # n-columns per stage-1 matmul (PSUM bank = 512 f32)
SCALE = 1.0 / 16.0   # 1/sqrt(K)

KT = K // 128        # 2 contraction tiles for stage 1
MT = M // 128        # 8 logits partition tiles / stage-2 contraction tiles
NCH = N // NCHUNK    # stage-1 chunks per row
NT = N // 128        # stage-2 n-tiles
OB = 4               # outT n-tiles batched per DMA

_cache = {}


def _build():
    from concourse import bacc
    import concourse.tile as tile
    from concourse import mybir
    from contextlib import ExitStack

    f32 = mybir.dt.float32
    bf16 = mybir.dt.bfloat16

    nc = bacc.Bacc("TRN2", target_bir_lowering=False, debug=False,
                   num_devices=NCORES)

    x_d = nc.dram_tensor("x", [K, N], bf16, kind="ExternalInput").ap()
    mkm_d = nc.dram_tensor("mem_km", [K, M], bf16, kind="ExternalInput").ap()
    mt1_d = nc.dram_tensor("mem_t1", [M, K + 1], bf16, kind="ExternalInput").ap()
    # outputs are stored bf16 on device (halves write traffic; the host
    # upcasts) — quantization adds ~1e-3 rel err, small next to the bf16
    # matmul error and far inside tolerance
    lg_d = nc.dram_tensor("logits", [M, N], bf16, kind="ExternalOutput").ap()
    ot_d = nc.dram_tensor("outT", [N, K], bf16, kind="ExternalOutput").ap()

    with tile.TileContext(nc) as tc, ExitStack() as ctx:
        const = ctx.enter_context(tc.tile_pool(name="const", bufs=1))

        x_sb = const.tile([128, KT, N], bf16)
        mkm_sb = const.tile([128, KT, M], bf16)
        mt1_sb = const.tile([128, MT, K + 1], bf16)
        x_re = x_d.rearrange("(t p) n -> p t n", p=128)
        # first x block rides HWDGE as f32 (starts earlier than SWDGE) and
        # DVE casts it; the rest streams via SWDGE with the f32->bf16 cast
        # fused into the DMA. mem bank loads share the HWDGE path.
        mkm_re = mkm_d.rearrange("(t p) m -> p t m", p=128)
        # HWDGE order is what the first matmul waits on: tiny first slice of
        # the memory bank, then the first x block, then the rest streaming
        nc.sync.dma_start(out=mkm_sb[:, :, 0:128], in_=mkm_re[:, :, 0:128])
        nc.sync.dma_start(out=x_sb[:, :, 0:512], in_=x_re[:, :, 0:512])
        nc.sync.dma_start(out=mkm_sb[:, :, 128:M], in_=mkm_re[:, :, 128:M])
        x_edges = [512, 1024, 2048, 3072, 4096]
        for lo, hi in zip(x_edges[:-1], x_edges[1:]):
            nc.sync.dma_start(out=x_sb[:, :, lo:hi], in_=x_re[:, :, lo:hi])
        nc.sync.dma_start(out=mt1_sb,
                          in_=mt1_d.rearrange("(t p) c -> p t c", p=128))

        psumL = ctx.enter_context(tc.tile_pool(name="psumL", bufs=3, space="PSUM"))
        psumO = ctx.enter_context(tc.tile_pool(name="psumO", bufs=2, space="PSUM"))
        lpool = ctx.enter_context(tc.tile_pool(name="lsb", bufs=2))
        epool = ctx.enter_context(tc.tile_pool(name="expl", bufs=1))
        opool = ctx.enter_context(tc.tile_pool(name="osb", bufs=2))
        rpool = ctx.enter_context(tc.tile_pool(name="rcp", bufs=4))

        # full exp(logits) stays resident: 8 x (128, 4096) bf16 = 64KB/partition
        expl = epool.tile([128, MT, N], bf16)
        ot_re = ot_d.rearrange("(g p) k -> p g k", p=128)

        HALVES = 2
        HN = N // HALVES          # columns per half
        HC = HN // NCHUNK         # stage-1 chunks per half
        HT = NT // HALVES         # stage-2 n-tiles per half

        LW = 2 * NCHUNK           # PSUM L spans 2 banks -> halves DVE/ACT ops
        NB = N // LW              # interleave blocks
        lsb = {}
        for h in range(HALVES):
            lsb_h = lpool.tile([128, MT, HN], bf16, tag="lsb")
            lsb[h] = lsb_h
        # Full interleave: each LW-wide block does stage 1 for all MT rows,
        # then immediately stage 2 for its LW//128 n-tiles — PE alternates
        # between the two matmul shapes with no phase barrier.
        for cc in range(NB):
            base = cc * LW
            h = (cc * LW) // HN
            nr = slice(base - h * HN, base - h * HN + LW)
            for mt in range(MT):
                ms = slice(mt * 128, (mt + 1) * 128)
                L = psumL.tile([128, LW], f32, tag="L")
                # kt-major so consecutive matmuls reuse the stationary weights
                for kt in range(KT):
                    for sub in range(LW // NCHUNK):
                        ns = slice(base + sub * NCHUNK, base + (sub + 1) * NCHUNK)
                        ps = slice(sub * NCHUNK, (sub + 1) * NCHUNK)
                        nc.tensor.matmul(L[:, ps], mkm_sb[:, kt, ms],
                                         x_sb[:, kt, ns],
                                         start=(kt == 0), stop=(kt == KT - 1))
                nc.vector.tensor_scalar_mul(lsb[h][:, mt, nr], L, SCALE)
                nc.scalar.activation(expl[:, mt, base:base + LW], L,
                                     mybir.ActivationFunctionType.Exp,
                                     scale=SCALE)
            if (cc + 1) * LW % HN == 0:
                hs = slice(h * HN, (h + 1) * HN)
                for mt in range(MT):
                    nc.sync.dma_start(out=lg_d[mt * 128:(mt + 1) * 128, hs],
                                      in_=lsb[h][:, mt, :])
            # ---- stage 2 for this block's n-tiles ----
            for g in range(LW // 128 // OB):
                osb = opool.tile([128, OB, K], bf16, tag="osb")
                for t in range(OB):
                    nt = cc * (LW // 128) + g * OB + t
                    ts_ = slice(nt * 128, (nt + 1) * 128)
                    O = psumO.tile([128, K + 1], f32, tag="O")
                    for mt in range(MT):
                        nc.tensor.matmul(O, expl[:, mt, ts_], mt1_sb[:, mt, :],
                                         start=(mt == 0), stop=(mt == MT - 1))
                    r = rpool.tile([128, 1], f32, tag="rcp")
                    nc.vector.reciprocal(r, O[:, K:K + 1])
                    # alternate the normalize between ACT and DVE to balance
                    if t % 2 == 0:
                        nc.scalar.mul(osb[:, t, :], O[:, 0:K], r)
                    else:
                        nc.vector.tensor_scalar_mul(osb[:, t, :], O[:, 0:K], r)
                g0 = cc * (LW // 128) + g * OB
                nc.sync.dma_start(out=ot_re[:, g0:g0 + OB, :], in_=osb)

    nc.compile()
    return nc


def _get_nc():
    if "nc" not in _cache:
        _cache["nc"] = _build()
    return _cache["nc"]


def _run(x, mem, trace=False, **kwargs):
    import ml_dtypes
    from concourse.bass_utils import run_bass_kernel_spmd

    nc = _get_nc()
    x = np.asarray(x, dtype=np.float32).reshape(B, K, N).astype(ml_dtypes.bfloat16)
    mem0 = np.asarray(mem, dtype=np.float32)[0]                    # (K, M)
    mkm = mem0.astype(ml_dtypes.bfloat16)
    mt1 = np.concatenate(
        [mem0.T, np.ones((M, 1), np.float32)], axis=1
    ).astype(ml_dtypes.bfloat16)                                   # (M, K+1)
    in_maps = [
        {"x": np.ascontiguousarray(x[b]), "mem_km": mkm, "mem_t1": mt1}
        for b in range(B)
    ]
    return run_bass_kernel_spmd(nc, in_maps, core_ids=list(range(NCORES)),
                                trace=trace, **kwargs)


def kernel(x, mask, mem):
    res = _run(x, mem, trace=False)
    logits = np.stack(
        [res.results[b]["logits"].astype(np.float32) for b in range(B)]
    )
    out = np.stack(
        [res.results[b]["outT"].astype(np.float32).T.reshape(K, HH, WW)
         for b in range(B)]
    )
    return out, logits
